# revision 2
# baseline (speedup 1.0000x reference)
"""CorefHead Trainium2 kernel.

Reference computation (B=64, S=512, H=1024, HID=512):
  emb_a = span_mean(bert, offsets[:,0:2])   # [B,H]
  emb_b = span_mean(bert, offsets[:,2:4])   # [B,H]
  emb_p = bert[b, offsets[:,4]]             # [B,H]
  x = concat([emb_a, emb_b, emb_p], -1)     # [B,3H]
  h = leaky_relu(batchnorm_eval(x @ W1 + b1), 0.01)
  out = h @ W2 + b2                         # [B,3]

Strategy: pure data parallel, batch sharded 8 ways (8 batches/core).
The kernel is HBM-stream-bound (~0.39 B/ns/core with all 8 cores
streaming), so the schedule is built around the arrival order of a
single continuous stream:
  - bert rows ship FIRST (both HWDGE rings), as fp8 e4m3 for long
    spans (quantization noise averages out over the span) and bf16 for
    short spans + pron rows. Rows are packed partition-major with MW=24
    mask columns (3 embeddings x 8 batches) per 128-row chunk, padded
    to a 32-col mask region so the chunk width (1056) is 16-divisible
    (DoubleRow AP constraint).
  - mm1 uses fp8 DoubleRow: one matmul contracts a PAIR of 128-row
    chunks ([128,2,24] stationary, [128,2,512] moving), 2x the
    column rate, so mm1 tracks the stream instead of lagging it.
  - W1 ships AFTER bert in consumption-ordered pieces (span rows fp8
    e3m4 x16 on ring0, pron rows bf16 on ring1 -- pron x-values are
    ~12x larger so pron W1 stays bf16 for accuracy), sized so mm2's
    hc-major consumption is co-paced with arrival and the tail piece
    is small.
  - PE clock pre-warm (HAM) runs on memset junk fed by the Vector
    engine (earliest-free engine) while the first DMAs issue.
  - Tail: bias matmul issued early, lrelu split in halves to overlap
    ACT with the PE transposes, b2 folded into the PSUM->SBUF output
    copy via ACT per-partition bias. Host gathers per-core [3, 8]
    outputs and undoes the batch permutation.
"""

import numpy as np

B, S, H = 64, 512, 1024
HID = 512
EPS = 1e-5
NCORES = 8
BPC = B // NCORES  # batches per core
NMC = 3 * BPC      # mask columns: (embedding e, batch slot b) -> e*BPC + b
MW = NMC           # mask width (cols 0:NMC of each chunk)
PADW = 32          # mask region padded so CW % 16 == 0 (DoubleRow AP rule)
CW = PADW + H      # chunk width: mask region + bert row

# Span rows >= T ship as fp8 e4m3 (DoubleRow-capable); shorter spans
# (and pron rows) ship bf16.
T_FP8 = 32
W1_SCALE = 16.0    # prescale for W1 span rows in fp8 e3m4
N_WARM = 5         # dummy matmuls to pre-warm the PE clock

TRACE = False
LAST_RESULT = None

_PROGRAM_CACHE: dict = {}

# W1 piece layout (consumption-ordered). Span pieces slice the
# [128, 16, HID] e3m4 tensor (k = hc*2+e); pron pieces slice the
# [128, 8, HID] bf16 tensor (k = hc).
W1S_PIECES = [(0, 4), (4, 4), (8, 4), (12, 2), (14, 1), (15, 1)]
W1P_PIECES = [(0, 2), (2, 2), (4, 2), (6, 1), (7, 1)]


def _build_program(nch8: int, nchb: int, bp: int):
    import concourse.bacc as bacc
    import concourse.tile as tile
    import concourse.mybir as mybir
    from concourse.bass import MemorySpace

    f32 = mybir.dt.float32
    bf = mybir.dt.bfloat16
    e4 = mybir.dt.float8e4
    w8 = mybir.dt.float8e3
    DR = mybir.MatmulPerfMode.DoubleRow

    nc = bacc.Bacc("TRN2", target_bir_lowering=False, debug=False,
                   num_devices=NCORES)

    HC = H // 128       # 8

    e3_d = nc.dram_tensor("e3buf", [128, nch8, CW], e4,
                          kind="ExternalInput").ap()
    bf_d = nc.dram_tensor("bfbuf", [bp, nchb, CW], bf,
                          kind="ExternalInput").ap()
    w1s_d = nc.dram_tensor("w1S", [128, 16, HID], w8,
                           kind="ExternalInput").ap()
    w1p_d = nc.dram_tensor("w1P", [128, 8, HID], bf,
                           kind="ExternalInput").ap()
    # cstA (bf16): [0:24]=ident24, [24:36]=W2 (4 chunks x 3), [36]=b2 rows 0:3
    # cstB (bf16, 1 partition): [0:512]=bn bias, [512:520]=ones
    cstA_d = nc.dram_tensor("cstA", [128, 37], bf, kind="ExternalInput").ap()
    cstB_d = nc.dram_tensor("cstB", [1, 523], bf, kind="ExternalInput").ap()
    sfac_d = nc.dram_tensor("sfac", [NMC, 1], f32, kind="ExternalInput").ap()
    out_d = nc.dram_tensor("out", [3, BPC], f32, kind="ExternalOutput").ap()

    with tile.TileContext(nc) as tc:
        with (
            tc.tile_pool(name="data", bufs=1) as data,
            tc.tile_pool(name="work", bufs=1) as work,
            tc.tile_pool(name="psum_x", bufs=1, space=MemorySpace.PSUM) as psx,
            tc.tile_pool(name="psum_t", bufs=4, space=MemorySpace.PSUM) as pst,
            tc.tile_pool(name="psum_h", bufs=1, space=MemorySpace.PSUM) as psh,
        ):
            # --- PE pre-warm on memset junk. Vector's preamble retires
            # earliest and DVE is otherwise idle here, so it feeds the
            # junk; GpSimd goes straight to the const DMAs.
            junk = work.tile([128, 512], bf, tag="junk")
            nc.vector.memset(junk, 0.0)
            scr = psh.tile([24, 512], f32, tag="scratch")
            for _ in range(N_WARM):
                nc.tensor.matmul(scr, junk[:, 0:24], junk, start=True,
                                 stop=True)

            # --- DMA issue. Consts ride the gpsimd SWDGE ring (tiny,
            # needed mid-kernel). Data: bert first on both HWDGE rings
            # (ring1 leads with the small bf16 buffer), then W1 pieces
            # in mm2 consumption order (span on ring0, pron on ring1).
            cstA_t = data.tile([128, 37], bf, tag="cstA")
            nc.gpsimd.dma_start(out=cstA_t, in_=cstA_d)
            cstB_t = data.tile([1, 523], bf, tag="cstB")
            nc.gpsimd.dma_start(out=cstB_t, in_=cstB_d)
            sfac_t = data.tile([NMC, 1], f32, tag="sfac")
            nc.gpsimd.dma_start(out=sfac_t, in_=sfac_d)

            ring0, ring1 = nc.sync, nc.scalar

            bf_t = data.tile([bp, nchb, CW], bf, tag="bfp")
            ring1.dma_start(out=bf_t, in_=bf_d)

            # e3 pieces: 2 chunks each (even, for DoubleRow pairing),
            # alternating rings starting with ring0.
            e3_ts = []
            for i, c0 in enumerate(range(0, nch8, 2)):
                t = data.tile([128, 2, CW], e4, tag=f"e3p{i}",
                              name=f"e3p{i}")
                (ring0 if i % 2 == 0 else ring1).dma_start(
                    out=t, in_=e3_d[:, c0:c0 + 2, :])
                e3_ts.append(t)

            w1s_ts = []
            for i, (k0, kn) in enumerate(W1S_PIECES):
                t = data.tile([128, kn, HID], w8, tag=f"w1s{i}",
                              name=f"w1s{i}")
                ring0.dma_start(out=t, in_=w1s_d[:, k0:k0 + kn, :])
                w1s_ts.append((k0, kn, t))
            w1p_ts = []
            for i, (k0, kn) in enumerate(W1P_PIECES):
                t = data.tile([128, kn, HID], bf, tag=f"w1p{i}",
                              name=f"w1p{i}")
                ring1.dma_start(out=t, in_=w1p_d[:, k0:k0 + kn, :])
                w1p_ts.append((k0, kn, t))

            def w1_slice(hc, e):
                if e < 2:
                    k = hc * 2 + e
                    for k0, kn, t in w1s_ts:
                        if k0 <= k < k0 + kn:
                            return t[:, k - k0, :]
                for k0, kn, t in w1p_ts:
                    if k0 <= hc < k0 + kn:
                        return t[:, hc - k0, :]
                raise AssertionError

            # ACT table loads for Identity/Lrelu, after the DMA issues
            jact = work.tile([128, 32], bf, tag="jact")
            nc.scalar.activation(jact, junk[:, 0:32],
                                 mybir.ActivationFunctionType.Identity,
                                 scale=2.0)
            nc.scalar.activation(jact, junk[:, 0:32],
                                 mybir.ActivationFunctionType.Lrelu,
                                 alpha=0.01)

            # --- mm1: x[24, 1024] += mask.T @ bert over chunks.
            # bf16 chunks first (they arrive earliest and run during
            # the clock ramp), then fp8 chunk pairs via DoubleRow.
            px0 = psx.tile([NMC, 512], f32, tag="px0")
            px1 = psx.tile([NMC, 512], f32, tag="px1")
            ph = psh.tile([BPC, HID], f32, tag="ph")

            for c in range(nchb):
                m = bf_t[:, c, 0:MW]
                nc.tensor.matmul(px0, m, bf_t[:, c, PADW:PADW + 512],
                                 start=(c == 0), stop=False)
                nc.tensor.matmul(px1, m, bf_t[:, c, PADW + 512:CW],
                                 start=(c == 0), stop=False)

            npair = nch8 // 2
            for i in range(npair):
                t = e3_ts[i]
                m = t[:, 0:2, 0:MW]
                nc.tensor.matmul(px0, m, t[:, 0:2, PADW:PADW + 512],
                                 start=False, stop=False, perf_mode=DR)
                nc.tensor.matmul(px1, m, t[:, 0:2, PADW + 512:CW],
                                 start=False, stop=(i == npair - 1),
                                 perf_mode=DR)
                if i == 1:
                    # BN bias into ph while consts have surely landed
                    nc.tensor.matmul(ph, cstB_t[0:1, 512:520],
                                     cstB_t[0:1, 0:512], start=True,
                                     stop=False)

            # --- x: scale by 1/len (fp32) + cast bf16, split ACT/DVE
            xsb = work.tile([NMC, H], bf, tag="xsb")
            nc.scalar.activation(xsb[:, 0:512], px0,
                                 mybir.ActivationFunctionType.Identity,
                                 scale=sfac_t)
            nc.vector.tensor_scalar_mul(xsb[:, 512:H], px1, sfac_t)

            # --- transposes up front (hidden under the W1 stream),
            # then mm2 hc-major, co-paced with W1 piece arrival:
            # h[8, 512] = x @ (W1*bn_s) + bn_bias
            ident24 = cstA_t[0:NMC, 0:NMC]
            xT = work.tile([128, HC, NMC], bf, tag="xT")
            for hc in range(HC):
                pT = pst.tile([128, NMC], bf, tag="pT")
                nc.tensor.transpose(pT, xsb[:, hc * 128:(hc + 1) * 128],
                                    ident24)
                nc.vector.tensor_copy(xT[:, hc, :], pT)
            for hc in range(HC):
                for e in range(3):
                    nc.tensor.matmul(
                        ph, xT[:, hc, e * BPC:(e + 1) * BPC], w1_slice(hc, e),
                        start=False, stop=(hc == HC - 1 and e == 2),
                    )

            # --- LeakyReLU on ACT in halves (overlaps PE transposes)
            y = work.tile([BPC, HID], bf, tag="y")
            ident8 = cstA_t[0:BPC, 0:BPC]
            yT_ps = [pst.tile([128, BPC], bf, tag="pT", name=f"yTp{mc}")
                     for mc in range(4)]
            yT_sb = [work.tile([128, BPC], bf, tag=f"yTs{mc}", name=f"yTs{mc}")
                     for mc in range(4)]
            for half in range(2):
                nc.scalar.activation(y[:, half * 256:(half + 1) * 256],
                                     ph[:, half * 256:(half + 1) * 256],
                                     mybir.ActivationFunctionType.Lrelu,
                                     alpha=0.01)
                for mc in (2 * half, 2 * half + 1):
                    nc.tensor.transpose(
                        yT_ps[mc], y[:, mc * 128:(mc + 1) * 128], ident8)
                    nc.vector.tensor_copy(yT_sb[mc], yT_ps[mc])

            # --- mm3: out[3, 8] = W2.T @ y.T; b2 folds into the copy
            oT = psx.tile([3, BPC], f32, tag="px0")
            for mc in range(4):
                nc.tensor.matmul(oT, cstA_t[:, 24 + 3 * mc:27 + 3 * mc],
                                 yT_sb[mc], start=(mc == 0), stop=(mc == 3))
            o_sb = work.tile([3, BPC], f32, tag="osb")
            nc.scalar.activation(o_sb, oT,
                                 mybir.ActivationFunctionType.Identity,
                                 bias=cstA_t[0:3, 36:37])
            nc.sync.dma_start(out=out_d, in_=o_sb)

    nc.compile()
    return nc


def _pack_rows(rows, masks, nch, np_dt, part=128):
    """rows: [N, H] fp32, masks: [N, NMC] fp32 -> [part, nch, CW] np_dt,
    partition-major (packed position i -> (p=i%part, c=i//part))."""
    N = rows.shape[0]
    buf = np.zeros((nch * part, CW), dtype=np.float32)
    if N:
        buf[:N, :MW] = masks
        buf[:N, PADW:] = rows
    return np.ascontiguousarray(
        buf.reshape(nch, part, CW).transpose(1, 0, 2)).astype(np_dt)


def _prep_core_inputs(bert, offs, w1_bufs, cstA, cstB, batch_idx,
                      nch8, nchb, bp):
    import ml_dtypes
    bf16 = ml_dtypes.bfloat16
    e4 = ml_dtypes.float8_e4m3
    f8_rows, f8_masks = [], []
    b16_rows, b16_masks = [], []
    sfac = np.ones((NMC, 1), dtype=np.float32)
    for slot, gb in enumerate(batch_idx):
        a0, a1, b0, b1_, p = (int(v) for v in offs[gb])
        spans = [(a0, a1, 0), (b0, b1_, 1)]
        long_spans = [s for s in spans if s[1] - s[0] + 1 >= T_FP8]
        short_spans = [s for s in spans if s[1] - s[0] + 1 < T_FP8]
        for (lo, hi, e) in spans:
            wsc = W1_SCALE if e < 2 else 1.0
            sfac[e * BPC + slot, 0] = 1.0 / ((hi - lo + 1) * wsc)
        if long_spans:
            lo = min(s[0] for s in long_spans)
            hi = max(s[1] for s in long_spans)
            pos = np.arange(lo, hi + 1)
            keep = np.zeros(len(pos), dtype=bool)
            m = np.zeros((len(pos), NMC), dtype=np.float32)
            for (s0, s1, e) in long_spans:
                sel = (pos >= s0) & (pos <= s1)
                keep |= sel
                m[sel, e * BPC + slot] = 1.0
            f8_rows.append(bert[gb, pos[keep]])
            f8_masks.append(m[keep])
        want = {}
        for (s0, s1, e) in short_spans:
            for r in range(s0, s1 + 1):
                want.setdefault(r, []).append(e)
        want.setdefault(p, []).append(2)
        if want:
            rs = sorted(want)
            m = np.zeros((len(rs), NMC), dtype=np.float32)
            for i, r in enumerate(rs):
                for e in want[r]:
                    m[i, e * BPC + slot] = 1.0
            b16_rows.append(bert[gb, rs])
            b16_masks.append(m)

    def cat(parts, w):
        return (np.concatenate(parts, axis=0) if parts
                else np.zeros((0, w), dtype=np.float32))

    in_map = {
        "e3buf": _pack_rows(cat(f8_rows, H), cat(f8_masks, NMC), nch8, e4),
        "bfbuf": _pack_rows(cat(b16_rows, H), cat(b16_masks, NMC), nchb, bf16,
                            part=bp),
        "cstA": cstA,
        "cstB": cstB,
        "sfac": sfac,
    }
    in_map.update(w1_bufs)
    return in_map


def _row_counts(offs):
    """Per-batch (fp8 rows, bf16 rows) under the T_FP8 split."""
    n8 = np.zeros(B, dtype=np.int64)
    nb = np.zeros(B, dtype=np.int64)
    for gb in range(B):
        a0, a1, b0, b1_, p = (int(v) for v in offs[gb])
        spans = [(a0, a1), (b0, b1_)]
        longs = [s for s in spans if s[1] - s[0] + 1 >= T_FP8]
        shorts = [s for s in spans if s[1] - s[0] + 1 < T_FP8]
        if longs:
            lo = min(s[0] for s in longs)
            hi = max(s[1] for s in longs)
            keep = np.zeros(hi - lo + 1, dtype=bool)
            for (s0, s1) in longs:
                keep[s0 - lo:s1 - lo + 1] = True
            n8[gb] = keep.sum()
        rows = set()
        for (s0, s1) in shorts:
            rows.update(range(s0, s1 + 1))
        rows.add(p)
        nb[gb] = len(rows)
    return n8, nb


def kernel(bert_outputs, offsets, W1, b1, gamma, beta, running_mean,
           running_var, W2, b2):
    import ml_dtypes
    bf16 = ml_dtypes.bfloat16
    e3 = ml_dtypes.float8_e3m4

    bert = np.ascontiguousarray(np.asarray(bert_outputs, dtype=np.float32))
    offs = np.asarray(offsets).astype(np.int64)
    W1 = np.asarray(W1, dtype=np.float32)
    b1 = np.asarray(b1, dtype=np.float32)
    gamma = np.asarray(gamma, dtype=np.float32)
    beta = np.asarray(beta, dtype=np.float32)
    rm = np.asarray(running_mean, dtype=np.float32)
    rv = np.asarray(running_var, dtype=np.float32)
    W2 = np.asarray(W2, dtype=np.float32)
    b2 = np.asarray(b2, dtype=np.float32)

    # Fold BN eval stats: bn(xW1 + b1) = x(W1*s) + ((b1 - mean)*s + beta)
    s = gamma / np.sqrt(rv + EPS)
    bias = (b1 - rm) * s + beta
    W1s = W1 * s[None, :]
    w1ehc = W1s.reshape(3, 8, 128, HID)  # [e, hc, p, n]
    w1_bufs = {
        # [p, hc*2+e, n] for span embeddings e in {0,1}, fp8 e3m4 x16
        "w1S": np.ascontiguousarray(
            w1ehc[:2].transpose(2, 1, 0, 3).reshape(128, 16, HID)
            * W1_SCALE).astype(e3),
        # [p, hc, n] for the pron embedding, bf16
        "w1P": np.ascontiguousarray(
            w1ehc[2].transpose(1, 0, 2)).astype(bf16),
    }

    cstA = np.zeros((128, 37), dtype=np.float32)
    cstA[:NMC, :NMC] = np.eye(NMC)
    cstA[:, 24:36] = W2.reshape(4, 128, 3).transpose(1, 0, 2).reshape(128, 12)
    cstA[0:3, 36] = b2
    cstA = cstA.astype(bf16)
    cstB = np.zeros((1, 523), dtype=np.float32)
    cstB[0, 0:512] = bias
    cstB[0, 512:520] = 1.0
    cstB = cstB.astype(bf16)

    # Greedy-balance batches across cores by shipped bytes (fp8 row =
    # CW bytes, bf16 row = 2*CW), capped at BPC batches per core
    n8, nb = _row_counts(offs)
    cost = n8 + 2 * nb
    order = np.argsort(-cost, kind="stable")
    core_rows8 = np.zeros(NCORES, dtype=np.int64)
    core_rowsb = np.zeros(NCORES, dtype=np.int64)
    core_batches = [[] for _ in range(NCORES)]
    for gb in order:
        load = core_rows8 + 2 * core_rowsb
        load[np.array([len(cb) >= BPC for cb in core_batches])] = 1 << 40
        c = int(np.argmin(load))
        core_batches[c].append(int(gb))
        core_rows8[c] += n8[gb]
        core_rowsb[c] += nb[gb]
    nch8 = max(2, int((core_rows8.max() + 127) // 128))
    nch8 += nch8 % 2  # even chunk count for DoubleRow pairs
    maxb = int(core_rowsb.max())
    if maxb <= 96:
        bp = max(32, (maxb + 31) // 32 * 32)
        nchb = 1
    else:
        bp = 128
        nchb = (maxb + 127) // 128

    key = (nch8, nchb, bp)
    if key not in _PROGRAM_CACHE:
        _PROGRAM_CACHE[key] = _build_program(nch8, nchb, bp)
    nc = _PROGRAM_CACHE[key]

    in_maps = [
        _prep_core_inputs(bert, offs, w1_bufs, cstA, cstB, core_batches[c],
                          nch8, nchb, bp)
        for c in range(NCORES)
    ]

    from concourse import bass_utils
    kwargs = {}
    if TRACE:
        kwargs = {"trace": True, "trace_cores": list(range(NCORES))}
    res = bass_utils.run_bass_kernel_spmd(nc, in_maps,
                                          core_ids=list(range(NCORES)),
                                          **kwargs)
    global LAST_RESULT
    LAST_RESULT = res

    out = np.empty((B, 3), dtype=np.float32)
    for c in range(NCORES):
        out[core_batches[c]] = res.results[c]["out"].T
    return out
